# revision 19
# baseline (speedup 1.0000x reference)
"""Trainium2 Bass kernel for nn_CoarseTransformer (B=2, N=1025, D=1024, 6 layers,
MQA + T5 rel-pos bias + GEGLU FF, sem/coarse logit heads).

Strategy: 8-way token-parallel SPMD. The padded 1032-slot sequence (1025 real
tokens + 7 dummies) splits into 8 chunks of 129 slots; each core processes its
chunk for BOTH batch items (258 token-columns) through all 6 layers. Weights
are replicated (streamed from each core's HBM); the only cross-core exchange is
a per-layer AllGather of the (tiny, multi-query) K/V projection. All activations
live transposed [feature, token] so weight matrices are the PE's stationary
operand in their natural [K, F] layout; fp32r matmuls everywhere except the
bf16 attention core. Softmax skips max-subtraction (|sim| < ~6 vs reference)
and folds the causal mask + T5 bias into a host-precomputed exp(bias)
multiplier; the softmax denominator comes free from an appended ones-column
on V. LN gamma/beta, the 1/sqrt(dh) query scale, and all head biases are
folded into the weight tensors host-side.

kernel(**inputs) -> (sem_logits [2,256,501], coarse_logits [2,769,1025])
"""
import numpy as np
import ml_dtypes

import concourse.bass as bass
import concourse.tile as tile
from concourse import bacc, mybir
from concourse.bass_utils import run_bass_kernel_spmd
from concourse.library_config import attn as attn_lib

F32 = mybir.dt.float32
F32R = mybir.dt.float32r
BF16 = mybir.dt.bfloat16
AF = mybir.ActivationFunctionType
OP = mybir.AluOpType

B = 2
NTOK = 1025
D = 1024
DEPTH = 6
H = 8
DH = 64
INNER = 512
FF = 2730
FFP = 2816
SEMV = 501
CBV = 1025
NQ = 3
CB = 1024
NUM_BUCKETS = 32
MAX_DIST = 128

NCORES = 8
PC = 129
SLOTS = NCORES * PC  # 1032
NC2 = 2 * PC  # 258
NJ = 9
DC = D // 128  # 8
FCH = FFP // 128  # 22
HEADV = 512 + 3 * 1152  # 3968
HF = HEADV // 128  # 31

_COMPILED = None


def f32v(ap):
    return ap.bitcast(F32)


def build(depth=DEPTH):
    nc = bacc.Bacc("TRN2", target_bir_lowering=False, debug=False, num_devices=NCORES)

    xt0 = nc.dram_tensor("xt0", [128, DC, NC2], F32R, kind="ExternalInput").ap()
    wqkv = nc.dram_tensor("wqkv", [depth, 128, DC, INNER + 2 * DH], F32R, kind="ExternalInput").ap()
    wo_ = nc.dram_tensor("wo_", [depth, 128, 4, D], F32R, kind="ExternalInput").ap()
    w1_ = nc.dram_tensor("w1_", [depth, 128, DC, 2 * FFP], F32R, kind="ExternalInput").ap()
    ff1b = nc.dram_tensor("ff1b", [depth, 128, 2 * FCH], F32, kind="ExternalInput").ap()
    w2_ = nc.dram_tensor("w2_", [depth, 128, FCH, D], F32R, kind="ExternalInput").ap()
    headw = nc.dram_tensor("headw", [128, DC, HEADV], F32R, kind="ExternalInput").ap()
    expb_in = nc.dram_tensor("expb_in", [128, NJ, H, PC], BF16, kind="ExternalInput").ap()
    ident_in = nc.dram_tensor("ident_in", [64, 64], BF16, kind="ExternalInput").ap()
    ones_col_in = nc.dram_tensor("ones_col_in", [128, 1], F32R, kind="ExternalInput").ap()
    ones_row_in = nc.dram_tensor("ones_row_in", [1, 128], F32R, kind="ExternalInput").ap()

    logits = nc.dram_tensor("logits", [128, HF, NC2], F32, kind="ExternalOutput").ap()
    xt_out = nc.dram_tensor("xt_out", [128, DC, NC2], F32, kind="ExternalOutput").ap()
    dbg_k = nc.dram_tensor("dbg_k", [DH, NJ * 128], F32, kind="ExternalOutput").ap()
    dbg_q = nc.dram_tensor("dbg_q", [DH, H, NC2], F32, kind="ExternalOutput").ap()
    dbg_vall = nc.dram_tensor("dbg_vall", [128, B, NJ, DH + 1], F32, kind="ExternalOutput").ap()
    dbg_qkv = nc.dram_tensor("dbg_qkv", [128, 5, NC2], F32, kind="ExternalOutput").ap()
    dbg_xn = nc.dram_tensor("dbg_xn", [128, DC, NC2], F32, kind="ExternalOutput").ap()
    dbg_st = nc.dram_tensor("dbg_st", [3, NC2], F32, kind="ExternalOutput").ap()
    dbg_xtin = nc.dram_tensor("dbg_xtin", [128, DC, NC2], F32, kind="ExternalOutput").ap()
    dbg_oT = nc.dram_tensor("dbg_oT", [128, 4, NC2], F32, kind="ExternalOutput").ap()
    dbg_den = nc.dram_tensor("dbg_den", [1, H, PC], F32, kind="ExternalOutput").ap()
    dbg_ex = nc.dram_tensor("dbg_ex", [128, 4, PC], F32, kind="ExternalOutput").ap()

    cc_in = [nc.dram_tensor(f"cc_in{l}", [128, NC2], BF16) for l in range(depth)]
    cc_out = [
        nc.dram_tensor(f"cc_out{l}", [NCORES, 128, NC2], BF16, addr_space="Shared")
        for l in range(depth)
    ]

    with tile.TileContext(nc) as tc:
        with (
            tc.tile_pool(name="res", bufs=1) as res,
            tc.tile_pool(name="wq_p", bufs=2) as wq_p,
            tc.tile_pool(name="wo_p", bufs=2) as wo_p,
            tc.tile_pool(name="w1_p", bufs=2) as w1_p,
            tc.tile_pool(name="w2_p", bufs=2) as w2_p,
            tc.tile_pool(name="act", bufs=1) as act_p,
            tc.tile_pool(name="sm", bufs=2) as sm_p,
            tc.tile_pool(name="bias", bufs=2) as bias_p,
        ):
            nc.gpsimd.load_library(attn_lib)

            xt = res.tile([128, DC, NC2], F32R, tag="xt")
            nc.sync.dma_start(out=xt[:], in_=xt0[:])
            expb = res.tile([128, NJ, H, PC], BF16, tag="expb")
            nc.sync.dma_start(out=expb[:], in_=expb_in[:])
            ident = res.tile([64, 64], BF16, tag="ident")
            nc.sync.dma_start(out=ident[:], in_=ident_in[:])

            ones_col = res.tile([128, 1], F32R, tag="ones_col")
            nc.sync.dma_start(out=ones_col[:], in_=ones_col_in[:])
            ones_row = res.tile([1, 128], F32R, tag="ones_row")
            nc.sync.dma_start(out=ones_row[:], in_=ones_row_in[:])
            eps_t = res.tile([1, 1], F32, tag="eps_t")
            nc.vector.memset(eps_t[:], 1e-5)

            kT0 = res.tile([DH, NJ * 128], BF16, tag="kT0")
            kT1 = res.tile([DH, NJ * 128], BF16, tag="kT1")
            vT0 = res.tile([DH, NJ * 128], BF16, tag="vT0")
            vT1 = res.tile([DH, NJ * 128], BF16, tag="vT1")
            kT = [kT0, kT1]
            vT = [vT0, vT1]
            for b in range(B):
                nc.vector.memset(kT[b][:, SLOTS:], 0.0)
                nc.vector.memset(vT[b][:, SLOTS:], 0.0)
            v_all = res.tile([128, B, NJ, DH + 1], BF16, tag="v_all")
            nc.vector.memset(v_all[:, :, :, DH:], 1.0)

            def layernorm(xin, dump=False):
                with tc.tile_pool(name="ps_ln", bufs=2, space="PSUM") as ps_ln:
                    xsq = act_p.tile([128, DC, NC2], F32R, tag="xsq")
                    nc.vector.tensor_tensor(out=xsq[:], in0=f32v(xin[:]), in1=f32v(xin[:]), op=OP.mult)
                    m_ps = ps_ln.tile([1, NC2], F32, tag="stat")
                    msq_ps = ps_ln.tile([1, NC2], F32, tag="stat")
                    for c in range(DC):
                        nc.tensor.matmul(out=m_ps[:], lhsT=ones_col[:], rhs=xin[:, c, :], start=(c == 0), stop=(c == DC - 1))
                    for c in range(DC):
                        nc.tensor.matmul(out=msq_ps[:], lhsT=ones_col[:], rhs=xsq[:, c, :], start=(c == 0), stop=(c == DC - 1))
                    m_s = sm_p.tile([1, NC2], F32R, tag="m_s")
                    with nc.allow_low_precision(reason="f32r rounding intended"):
                        nc.vector.tensor_copy(out=m_s[:], in_=m_ps[:])
                    var_s = sm_p.tile([1, NC2], F32, tag="var_s")
                    nc.vector.tensor_tensor(out=var_s[:], in0=f32v(m_s[:]), in1=f32v(m_s[:]), op=OP.mult)
                    nc.vector.tensor_tensor(out=var_s[:], in0=msq_ps[:], in1=var_s[:], op=OP.subtract)
                    std_s = sm_p.tile([1, NC2], F32, tag="std_s")
                    nc.scalar.activation(out=std_s[:], in_=var_s[:], func=AF.Sqrt, bias=eps_t[:])
                    rs_s = sm_p.tile([1, NC2], F32R, tag="rs_s")
                    with nc.allow_low_precision(reason="f32r rounding intended"):
                        nc.vector.reciprocal(out=rs_s[:], in_=std_s[:])
                    rs_b = ps_ln.tile([128, NC2], F32, tag="bcast")
                    nc.tensor.matmul(out=rs_b[:], lhsT=ones_row[:], rhs=rs_s[:], start=True, stop=True)
                    m_b = ps_ln.tile([128, NC2], F32, tag="bcast")
                    nc.tensor.matmul(out=m_b[:], lhsT=ones_row[:], rhs=m_s[:], start=True, stop=True)
                    if dump:
                        nc.gpsimd.dma_start(out=dbg_st[0:1, :], in_=m_s[:])
                        nc.gpsimd.dma_start(out=dbg_st[1:2, :], in_=var_s[:])
                        nc.gpsimd.dma_start(out=dbg_st[2:3, :], in_=f32v(rs_s[:]))
                    xn = act_p.tile([128, DC, NC2], F32R, tag="xn")
                    nc.vector.tensor_tensor(
                        out=xn[:],
                        in0=f32v(xin[:]),
                        in1=m_b[:, None, :].broadcast_to([128, DC, NC2]),
                        op=OP.subtract,
                    )
                    nc.vector.tensor_tensor(
                        out=xn[:],
                        in0=f32v(xn[:]),
                        in1=rs_b[:, None, :].broadcast_to([128, DC, NC2]),
                        op=OP.mult,
                    )
                return xn

            for l in range(depth):
                with nc.named_scope(f"L{l}_ln1"):
                    if l == 0:
                        nc.gpsimd.dma_start(out=dbg_xtin[:], in_=f32v(xt[:]))
                    xn = layernorm(xt, dump=(l == 0))
                    if l == 0:
                        nc.gpsimd.dma_start(out=dbg_xn[:], in_=f32v(xn[:]))

                with nc.named_scope(f"L{l}_qkv"), tc.tile_pool(name="ps_qkv", bufs=3, space="PSUM") as psq:
                    qkv_s = act_p.tile([128, 5, NC2], BF16, tag="qkv_s")
                    for f in range(5):
                        wt = wq_p.tile([128, DC, 128], F32R, tag="wq_t")
                        nc.sync.dma_start(out=wt[:], in_=wqkv[l][:, :, bass.ts(f, 128)])
                        ps = psq.tile([128, NC2], F32, tag="lin")
                        for c in range(DC):
                            nc.tensor.matmul(out=ps[:], lhsT=wt[:, c, :], rhs=xn[:, c, :], start=(c == 0), stop=(c == DC - 1))
                        nc.scalar.activation(out=qkv_s[:, f, :], in_=ps[:], func=AF.Copy)

                with nc.named_scope(f"L{l}_ag"), tc.tile_pool(name="ps_vt", bufs=2, space="PSUM") as psv:
                    nc.sync.dma_start(out=cc_in[l].ap(), in_=qkv_s[:, 4, :])
                    nc.gpsimd.collective_compute(
                        "AllGather",
                        OP.bypass,
                        replica_groups=[list(range(NCORES))],
                        ins=[cc_in[l].ap()],
                        outs=[cc_out[l].ap()],
                    )
                    for b in range(B):
                        nc.sync.dma_start(
                            out=kT[b][:, 0:SLOTS].rearrange("f (c r) -> f c r", c=NCORES),
                            in_=cc_out[l].ap()[:, 0:DH, bass.ts(b, PC)].rearrange("c f r -> f c r"),
                        )
                        nc.sync.dma_start(
                            out=vT[b][:, 0:SLOTS].rearrange("f (c r) -> f c r", c=NCORES),
                            in_=cc_out[l].ap()[:, DH : 2 * DH, bass.ts(b, PC)].rearrange("c f r -> f c r"),
                        )
                    # q rearrange to [64, H, NC2]: head h = 2t+u at (src part 64u+p, tile t)
                    qT2 = act_p.tile([DH, H, NC2], BF16, tag="qT2")
                    for u in range(2):
                        nc.sync.dma_start(out=qT2[:, u::2, :], in_=qkv_s[bass.ts(u, DH), 0:4, :])
                    for b in range(B):
                        for j in range(NJ):
                            tp = psv.tile([128, DH], BF16, tag="vt")
                            nc.tensor.transpose(
                                out=tp[:], in_=vT[b][:, bass.ts(j, 128)], identity=ident[:]
                            )
                            nc.vector.tensor_copy(out=v_all[:, b, j, 0:DH], in_=tp[:])

                if l == 0:
                    nc.gpsimd.dma_start(out=dbg_k[:], in_=kT[0][:])
                    nc.gpsimd.dma_start(out=dbg_q[:], in_=qT2[:])
                    nc.gpsimd.dma_start(out=dbg_vall[:], in_=v_all[:])
                    nc.gpsimd.dma_start(out=dbg_qkv[:], in_=qkv_s[:])
                with nc.named_scope(f"L{l}_attn"), \
                        tc.tile_pool(name="ps_av", bufs=1, space="PSUM") as psav, \
                        tc.tile_pool(name="ps_sim", bufs=2, space="PSUM") as pssim:
                    oT = act_p.tile([128, 4, NC2], F32R, tag="oT")
                    for b in range(B):
                        av = psav.tile([DH + 1, H, 256], F32, tag="av")
                        exs = act_p.tile([128, NJ, H, PC], BF16, tag="exs")
                        for j in range(NJ):
                            for g in range(2):
                                sim = pssim.tile([128, 4, 256], F32, tag="sim")
                                for hh in range(4):
                                    h = 4 * g + hh
                                    nc.tensor.matmul(
                                        out=sim[:, hh, 0:PC],
                                        lhsT=kT[b][:, bass.ts(j, 128)],
                                        rhs=qT2[:, h, bass.ts(b, PC)],
                                        start=True,
                                        stop=True,
                                    )
                                nc.scalar.activation(out=exs[:, j, bass.ts(g, 4), :], in_=sim[:, :, 0:PC], func=AF.Exp)
                                nc.vector.tensor_tensor(
                                    out=exs[:, j, bass.ts(g, 4), :],
                                    in0=exs[:, j, bass.ts(g, 4), :],
                                    in1=expb[:, j, bass.ts(g, 4), :],
                                    op=OP.mult,
                                )
                                if l == 0 and b == 0 and j == 0 and g == 0:
                                    nc.gpsimd.dma_start(out=dbg_ex[:], in_=exs[:, 0, 0:4, :])
                        # head-outer accumulation: a head finishes its bank before
                        # the bank-sharing neighbor's start=True clears has_written
                        for h in range(H):
                            for j in range(NJ):
                                nc.tensor.matmul(
                                    out=av[:, h, 0:PC],
                                    lhsT=v_all[:, b, j, :],
                                    rhs=exs[:, j, h, :],
                                    start=(j == 0),
                                    stop=(j == NJ - 1),
                                )
                        if l == 0 and b == 0:
                            den_s = sm_p.tile([1, H, PC], F32, tag="den_s")
                            nc.vector.tensor_copy(out=den_s[:], in_=av[DH : DH + 1, :, 0:PC])
                            nc.gpsimd.dma_start(out=dbg_den[:], in_=den_s[:])
                        rec = sm_p.tile([1, H, PC], F32R, tag="rec")
                        with nc.allow_low_precision(reason="f32r rounding intended"):
                            nc.vector.reciprocal(out=rec[:], in_=av[DH : DH + 1, :, 0:PC])
                        rb = sm_p.tile([DH, H, PC], F32R, tag="rb")
                        nc.gpsimd.partition_broadcast(f32v(rb[:]), f32v(rec[:]))
                        for u in range(2):
                            nc.vector.tensor_tensor(
                                out=oT[bass.ts(u, DH), :, bass.ts(b, PC)],
                                in0=av[0:DH, u::2, 0:PC],
                                in1=f32v(rb[:, u::2, :]),
                                op=OP.mult,
                            )

                if l == 0:
                    nc.gpsimd.dma_start(out=dbg_oT[:], in_=f32v(oT[:]))
                with nc.named_scope(f"L{l}_wo"), tc.tile_pool(name="ps_wo", bufs=3, space="PSUM") as psw:
                    for f in range(DC):
                        wt = wo_p.tile([128, 4, 128], F32R, tag="wo_t")
                        nc.sync.dma_start(out=wt[:], in_=wo_[l][:, :, bass.ts(f, 128)])
                        ps = psw.tile([128, NC2], F32, tag="lin")
                        for c in range(4):
                            nc.tensor.matmul(out=ps[:], lhsT=wt[:, c, :], rhs=oT[:, c, :], start=(c == 0), stop=(c == 3))
                        nc.vector.tensor_tensor(out=xt[:, f, :], in0=ps[:], in1=f32v(xt[:, f, :]), op=OP.add)

                with nc.named_scope(f"L{l}_ln2"):
                    xn2 = layernorm(xt)

                with nc.named_scope(f"L{l}_ff"), tc.tile_pool(name="ps_ff", bufs=4, space="PSUM") as psf:
                    fb = bias_p.tile([128, 2 * FCH], F32, tag="fb")
                    nc.sync.dma_start(out=fb[:], in_=ff1b[l])
                    ff_s = act_p.tile([128, FCH, NC2], F32R, tag="ff_s")
                    for i in range(FCH):
                        w1pair = w1_p.tile([128, DC, 256], F32R, tag="w1_t")
                        nc.sync.dma_start(out=w1pair[:, :, 0:128], in_=w1_[l][:, :, bass.ts(i, 128)])
                        nc.sync.dma_start(out=w1pair[:, :, 128:256], in_=w1_[l][:, :, FFP + i * 128 : FFP + (i + 1) * 128])
                        gate_ps = psf.tile([128, NC2], F32, tag="lin")
                        for c in range(DC):
                            nc.tensor.matmul(out=gate_ps[:], lhsT=w1pair[:, c, 128:256], rhs=xn2[:, c, :], start=(c == 0), stop=(c == DC - 1))
                        gelu_s = sm_p.tile([128, NC2], F32R, tag="gelu_s")
                        nc.scalar.activation(out=gelu_s[:], in_=gate_ps[:], func=AF.Gelu, bias=fb[:, FCH + i : FCH + i + 1])
                        a_ps = psf.tile([128, NC2], F32, tag="lin")
                        for c in range(DC):
                            nc.tensor.matmul(out=a_ps[:], lhsT=w1pair[:, c, 0:128], rhs=xn2[:, c, :], start=(c == 0), stop=(c == DC - 1))
                        nc.vector.scalar_tensor_tensor(
                            out=ff_s[:, i, :],
                            in0=a_ps[:],
                            scalar=fb[:, i : i + 1],
                            in1=f32v(gelu_s[:]),
                            op0=OP.add,
                            op1=OP.mult,
                        )
                    for f in range(DC):
                        wt = w2_p.tile([128, FCH, 128], F32R, tag="w2_t")
                        nc.sync.dma_start(out=wt[:], in_=w2_[l][:, :, bass.ts(f, 128)])
                        ps = psf.tile([128, NC2], F32, tag="lin")
                        for c in range(FCH):
                            nc.tensor.matmul(out=ps[:], lhsT=wt[:, c, :], rhs=ff_s[:, c, :], start=(c == 0), stop=(c == FCH - 1))
                        nc.vector.tensor_tensor(out=xt[:, f, :], in0=ps[:], in1=f32v(xt[:, f, :]), op=OP.add)

            nc.sync.dma_start(out=xt_out[:], in_=f32v(xt[:]))
            with nc.named_scope("final"), tc.tile_pool(name="ps_hd", bufs=3, space="PSUM") as psh:
                xnf = layernorm(xt)
                for f in range(HF):
                    wt = wo_p.tile([128, DC, 128], F32R, tag="hw_t")
                    nc.sync.dma_start(out=wt[:], in_=headw[:, :, bass.ts(f, 128)])
                    ps = psh.tile([128, NC2], F32, tag="lin")
                    for c in range(DC):
                        nc.tensor.matmul(out=ps[:], lhsT=wt[:, c, :], rhs=xnf[:, c, :], start=(c == 0), stop=(c == DC - 1))
                    hs = sm_p.tile([128, NC2], F32, tag="hs")
                    nc.scalar.activation(out=hs[:], in_=ps[:], func=AF.Copy)
                    nc.sync.dma_start(out=logits[:, f, :], in_=hs[:])

    nc.compile()
    return nc


# ============================ host side ============================


def _bucket(d):
    d = np.asarray(d)
    max_exact = NUM_BUCKETS // 2
    is_small = d < max_exact
    d_safe = np.maximum(d, max_exact).astype(np.float64)
    val_large = max_exact + (
        np.log(d_safe / max_exact) / np.log(MAX_DIST / max_exact) * (NUM_BUCKETS - max_exact)
    ).astype(np.int32)
    val_large = np.minimum(val_large, NUM_BUCKETS - 1)
    return np.where(is_small, d, val_large)


def prepare_inputs(inputs, depth=DEPTH):
    ins = {k: np.asarray(v) for k, v in inputs.items()}
    f32 = np.float32

    sem_ids = ins["semantic_token_ids"]
    coarse_ids = ins["coarse_token_ids"]
    tokens = np.zeros((B, SLOTS, D), dtype=f32)
    tokens[:, 0] = ins["start_token"]
    tokens[:, 1 : 1 + 256] = ins["sem_emb"][sem_ids]
    cids = coarse_ids + np.tile(np.arange(NQ) * CB, 256)[None, :]
    tokens[:, 257:NTOK] = ins["coarse_emb"][cids]

    rel = ins["rel_emb"]
    rb = rel[_bucket(np.arange(NTOK))]  # [NTOK, H]
    s_idx = np.arange(SLOTS)
    expb_all = np.zeros((NCORES, 128, NJ, H, PC), dtype=np.float32)
    for core in range(NCORES):
        qs = PC * core + np.arange(PC)
        dmat = qs[None, :] - s_idx[:, None]  # [SLOTS, PC]
        valid = (dmat >= 0) & (s_idx[:, None] <= 1024) & (qs[None, :] <= 1024)
        eb = np.exp(rb[np.clip(dmat, 0, NTOK - 1)])  # [SLOTS, PC, H]
        eb = np.where(valid[:, :, None], eb, 0.0)
        dummy_q = qs > 1024
        if dummy_q.any():
            eb[:, dummy_q, :] = 0.0
            eb[0, dummy_q, :] = 1.0
        ebp = np.zeros((NJ * 128, PC, H), dtype=eb.dtype)
        ebp[0:SLOTS] = eb
        expb_all[core] = ebp.reshape(NJ, 128, PC, H).transpose(1, 0, 3, 2)
    expb_all = expb_all.astype(ml_dtypes.bfloat16)

    def to_tiles(w):
        K, F = w.shape
        return np.ascontiguousarray(w.reshape(K // 128, 128, F).transpose(1, 0, 2)).astype(f32)

    wqkv_l, wo_l, w1_l, ff1b_l, w2_l = [], [], [], [], []
    scale = DH ** -0.5
    for l in range(depth):
        g, bb = ins["attn_g"][l].astype(np.float64), ins["attn_b"][l].astype(np.float64)
        wq = ins["wq"][l].astype(np.float64) * scale
        wkv = ins["wkv"][l].astype(np.float64)
        wcat = np.concatenate([wq, wkv], axis=1)
        wqkv_l.append(to_tiles((g[:, None] * wcat).astype(f32)))
        qv = bb @ wcat
        assert np.abs(qv).max() < 1e-10, "attn_b nonzero: device kernel drops qkv bias"
        wo_l.append(to_tiles(ins["wo"][l]))

        g2, b2 = ins["ff_g"][l].astype(np.float64), ins["ff_b"][l].astype(np.float64)
        w1 = ins["w1"][l].astype(np.float64)
        w1f = g2[:, None] * w1
        b1v = b2 @ w1
        w1p = np.zeros((D, 2 * FFP), dtype=f32)
        w1p[:, 0:FF] = w1f[:, 0:FF]
        w1p[:, FFP : FFP + FF] = w1f[:, FF : 2 * FF]
        b1p = np.zeros(2 * FFP, dtype=f32)
        b1p[0:FF] = b1v[0:FF]
        b1p[FFP : FFP + FF] = b1v[FF:]
        w1_l.append(to_tiles(w1p))
        ff1b_l.append(b1p.reshape(2 * FCH, 128).T.astype(f32))
        w2p = np.zeros((FFP, D), dtype=f32)
        w2p[0:FF] = ins["w2"][l]
        w2_l.append(to_tiles(w2p))

    gf, bf = ins["final_g"].astype(np.float64), ins["final_b"].astype(np.float64)
    hw = np.zeros((D, HEADV), dtype=np.float64)
    hw[:, 0:SEMV] = ins["sem_w"]
    for q in range(NQ):
        hw[:, 512 + 1152 * q : 512 + 1152 * q + CBV] = ins["coarse_w"][q].T
    hb = bf @ hw
    hb[0:SEMV] += ins["sem_b"]
    assert np.abs(hb).max() < 1e-10, "final_b/sem_b nonzero: device kernel drops head bias"
    headw_t = to_tiles((gf[:, None] * hw).astype(f32))

    shared = {
        "wqkv": np.ascontiguousarray(np.stack(wqkv_l)),
        "wo_": np.ascontiguousarray(np.stack(wo_l)),
        "w1_": np.ascontiguousarray(np.stack(w1_l)),
        "ff1b": np.ascontiguousarray(np.stack(ff1b_l)),
        "w2_": np.ascontiguousarray(np.stack(w2_l)),
        "headw": headw_t,
        "ident_in": np.eye(64, dtype=ml_dtypes.bfloat16),
        "ones_col_in": np.full((128, 1), 1.0 / D, dtype=np.float32),
        "ones_row_in": np.ones((1, 128), dtype=np.float32),
    }
    in_maps = []
    for core in range(NCORES):
        sl = tokens[:, PC * core : PC * (core + 1)]  # [B, PC, D]
        x = sl.transpose(0, 2, 1).reshape(B, DC, 128, PC)
        xt0_c = np.ascontiguousarray(x.transpose(2, 1, 0, 3).reshape(128, DC, NC2)).astype(f32)
        m = dict(shared)
        m["xt0"] = xt0_c
        m["expb_in"] = np.ascontiguousarray(expb_all[core])
        in_maps.append(m)
    return in_maps


def assemble_outputs(results):
    sem = np.zeros((B, 256, SEMV), dtype=np.float32)
    coarse = np.zeros((B, 769, CBV), dtype=np.float32)
    L = np.stack([results[c]["logits"] for c in range(NCORES)])  # [8, 128, HF, NC2]
    Lr = L.transpose(0, 3, 2, 1).reshape(NCORES, NC2, HEADV)  # [core, col, v]
    for b in range(B):
        idx = np.arange(256)
        cores, rs = np.divmod(idx, PC)
        sem[b] = Lr[cores, b * PC + rs, 0:SEMV]
        idx = np.arange(769) + 256
        cores, rs = np.divmod(idx, PC)
        for q in range(NQ):
            mask = (np.arange(769) % 3) == q
            coarse[b, mask] = Lr[cores[mask], b * PC + rs[mask], 512 + 1152 * q : 512 + 1152 * q + CBV]
    return sem, coarse


def kernel(**inputs):
    global _COMPILED
    if _COMPILED is None:
        _COMPILED = build()
    in_maps = prepare_inputs(inputs)
    res = run_bass_kernel_spmd(_COMPILED, in_maps, list(range(NCORES)))
    return assemble_outputs(res.results)
